# revision 43
# baseline (speedup 1.0000x reference)
"""KimiMoEGate (sigmoid scoring, group-limited top-k) on 8 Trainium2 cores.

Strategy (hardcoded for hidden_states [4,4096,2048], weight [256,2048]):
  - Token-parallel: 16384 tokens sharded 2048/core across 8 cores.
  - x is uploaded as int16 fixed-point (2 B/elem — half of fp32): host sends
    xi = round(x * s) with s = 32700/max|x|. The device splits xi exactly into
    fp16 hi/lo (xh = f16(xi), xl = xi - xh; both exact since |xi| < 2^15 and
    the residual is a small integer), then computes router logits on PE as
    2 fp16 passes (xh*wh + xh*wl) plus one fp8 DoubleRow pass (xl*wh8 —
    xl's small integers are exact in fp8e4, and DoubleRow packs two
    h-chunks per matmul at 2 MACs/cell/cycle), accumulated in fp32 PSUM.
  - W (scaled by 1024, split into fp16 hi/lo on host) is NOT replicated over
    the wire: each core uploads 2 of the 16 h-chunks (262KB instead of 2.1MB)
    and the full slab is assembled on-device with a single fused AllGather
    into Shared HBM scratch.
  - bias and the runtime sigmoid scale 2^-10/s ride a single [1, 257] row,
    broadcast across partitions on-device with a K=1 matmul against ones.
  - ACT does the i16->f16 hi split + sigmoid; DVE does the lo split and the
    group-limited top-k (reduce_max / match_replace / max_with_indices).
  - x rides the wire in natural [T, H] layout; the XBAR DMA transposes each
    tile on the way into SBUF (h_inner on partitions), so the host does no
    transpose at all.
  - The device returns the top-8 indices (u8), the top-8 BIASED scores m8
    (f16) and one per-token f32 "minimum boundary gap" = min(top-4 group
    cut, 8-vs-9 cut, adjacent top-8 gaps). The host recovers weights as
    wt_j = m8_j - bias[idx_j] and normalizes (wt / (sum + 1e-20) * 2.5);
    tokens whose minimum gap sits within the int16 quantization noise
    (~2% of tokens) are recomputed exactly on the host, making the result
    match the reference almost exactly.
  - PE schedule: the first 8 tiles run chunks 0-7 as eight concurrent PSUM
    accumulation groups, then finish interleaved with the remaining tiles'
    full runs; splits are prefetched ahead and outputs DMA'd per tile, so
    only the last tile's short chain trails the PE.
"""

import numpy as np

from concourse import bacc, bass_utils
import concourse.mybir as mybir
from concourse.tile import TileContext

F16 = mybir.dt.float16
F32 = mybir.dt.float32
F8 = mybir.dt.float8e4
U16 = mybir.dt.uint16
U8 = mybir.dt.uint8
I16 = mybir.dt.int16
AF = mybir.ActivationFunctionType
ALU = mybir.AluOpType
AX = mybir.AxisListType

N_CORES = 8
N_GROUP = 8
EXP_PER_GROUP = 32
E = 256
H = 2048
H_CHUNKS = 16  # 2048 / 128
T_TOTAL = 16384
T_CORE = T_TOTAL // N_CORES
N_TILES = T_CORE // 128  # 16


def build_kernel(nc, n_tiles=N_TILES):
    # natural token-major layout; the XBAR DMA transposes tiles on the way in
    xi = nc.dram_tensor("xi", [n_tiles, 128, H], I16, kind="ExternalInput").ap()
    # per-core W slice: [:, 0, :] = h-chunk c, [:, 1, :] = h-chunk 8+c
    # (chunk layout [128p(h_inner), 2E] with cols 0:256 = W_hi, 256:512 = W_lo)
    whl_part = nc.dram_tensor("whl_part", [128, 2, 2 * E], F16, kind="ExternalInput").ap()
    # bias row + sigmoid scale: bc[0, :256] = bias, bc[0, 256] = 2^-10/s
    bc = nc.dram_tensor("bc", [1, E + 1], F32, kind="ExternalInput").ap()
    idx_out = nc.dram_tensor("idx_out", [n_tiles, 128, 8], U8, kind="ExternalOutput").ap()
    m8_out = nc.dram_tensor("m8_out", [n_tiles, 128, 8], F16, kind="ExternalOutput").ap()
    # minimum routing-boundary gap per token, for host-side near-tie repair
    gap_out = nc.dram_tensor("gap_out", [n_tiles, 128, 1], F32, kind="ExternalOutput").ap()

    with TileContext(nc) as tc:
        with (
            tc.tile_pool(name="const", bufs=1) as cpool,
            tc.tile_pool(name="xin", bufs=4) as xipool,
            tc.tile_pool(name="xsplit", bufs=12) as xpool,
            tc.tile_pool(name="work", bufs=3) as wpool,
            tc.tile_pool(name="psum", bufs=8, space="PSUM") as ppool,
            tc.tile_pool(name="dram", bufs=1, space="DRAM") as dpool,
        ):
            # ---- W assembly: two pipelined AllGathers (chunks 0-7, 8-15) ----
            whl_sb = cpool.tile([128, H_CHUNKS, 2 * E], F16)
            # single fused gather: one collective launch for the whole 2.1MB
            # (two serialized collectives pay the ~55us launch+latency twice)
            gall = dpool.tile([128, 2, 2 * E], F16)
            goutall = nc.dram_tensor("goutall", [N_CORES, 128, 2, 2 * E], F16,
                                     kind="Internal", addr_space="Shared").ap()
            nc.gpsimd.dma_start(gall[:], whl_part[:])
            nc.gpsimd.collective_compute(
                "AllGather", ALU.bypass,
                replica_groups=[list(range(N_CORES))],
                ins=[gall.opt()], outs=[goutall.opt()])

            # ---- bias/scale broadcast across partitions via K=1 matmul ----
            bc_sb = cpool.tile([1, E + 1], F32)
            nc.sync.dma_start(bc_sb, bc)
            ones = cpool.tile([1, 128], F32)
            nc.gpsimd.memset(ones, 1.0)
            bias_full = cpool.tile([128, E], F32)
            ps_b = ppool.tile([128, E], F32, tag="ps")
            nc.tensor.matmul(ps_b, ones, bc_sb[:, :E], start=True, stop=True)
            nc.vector.tensor_copy(bias_full, ps_b)
            bias_sb = bias_full[:, :E]
            sv_sb = cpool.tile([128, 1], F32)
            ps_v = ppool.tile([128, 1], F32, tag="ps")
            nc.tensor.matmul(ps_v, ones, bc_sb[:, E:E + 1], start=True, stop=True)
            nc.vector.tensor_copy(sv_sb, ps_v)
            sv_ap = sv_sb[:, 0:1]

            # Preload the sigmoid activation table during the initial DMA
            # dead time so it isn't loaded mid-pipeline.
            dummy = cpool.tile([128, 1], F32)
            dummy2 = cpool.tile([128, 1], F32)
            nc.gpsimd.memset(dummy, 0.0)
            nc.scalar.activation(dummy2, dummy, AF.Sigmoid)

            def load_split(i):
                # exact split: xh = f16(xi) (RNE), xl = xi - xh (small int,
                # exact in f16)
                xi_sb = xipool.tile([128, H_CHUNKS, 128], I16, tag="xi")
                nc.sync.dma_start_transpose(xi_sb, xi[i])
                xh_sb = xpool.tile([128, H_CHUNKS, 128], F16, tag="xh")
                nc.scalar.activation(xh_sb, xi_sb, AF.Copy)
                # xl is a small integer (|xl| <= 8) — exact in fp8e4, which
                # lets the xl pass run as DoubleRow fp8 matmuls
                xl_sb = xpool.tile([128, H_CHUNKS, 128], F8, tag="xl")
                nc.vector.tensor_sub(xl_sb, xi_sb, xh_sb)
                return xh_sb, xl_sb

            # splits for the first HALF tiles are issued BEFORE the whl SBUF
            # loads: their xi DMAs are ready immediately and run during the
            # gathers instead of queueing behind DMA ops that wait on them.
            HALF = min(8, n_tiles)
            splits = {i: load_split(i) for i in range(HALF)}
            # fp8 copy of W_hi for the DoubleRow xl pass (one-time, post-gather)
            wh8_sb = cpool.tile([128, H_CHUNKS, E], F8)
            for ho in range(H_CHUNKS):
                nc.sync.dma_start(whl_sb[:, ho], goutall[ho % 8][:, ho // 8, :])
                nc.vector.tensor_copy(wh8_sb[:, ho], whl_sb[:, ho, :E])

            def mm_sweep(ps, xh_sb, xl_sb, ho_lo, ho_hi, first, last):
                for ho in range(ho_lo, ho_hi):
                    nc.tensor.matmul(ps, xh_sb[:, ho, :], whl_sb[:, ho, :E],
                                     start=(first and ho == ho_lo), stop=False)
                    nc.tensor.matmul(ps, xh_sb[:, ho, :], whl_sb[:, ho, E:],
                                     start=False, stop=False)
                # xl pass: fp8 DoubleRow, two h-chunks per matmul
                for ho in range(ho_lo, ho_hi, 2):
                    nc.tensor.matmul(ps, xl_sb[:, ho:ho + 2, :],
                                     wh8_sb[:, ho:ho + 2, :],
                                     perf_mode=mybir.MatmulPerfMode.DoubleRow,
                                     start=False,
                                     stop=(last and ho == ho_hi - 2))

            # sweep 1: first HALF tiles x chunks 0..7 — PSUM holds 8 open
            # accumulation groups, so these run as soon as gather A lands,
            # entirely under gather B.
            pss = {}
            for i in range(HALF):
                ps_i = ppool.tile([128, E], F32, tag="ps", name=f"ps{i}")
                pss[i] = ps_i
                mm_sweep(ps_i, *splits[i], 0, H_CHUNKS // 2, True, False)

            for j in range(HALF, min(HALF + 2, n_tiles)):
                splits[j] = load_split(j)

            def finish_tile(i, ps):
                # scores = sigmoid(logits); psum holds 1024*s*logits
                scores = wpool.tile([128, E], F32, tag="scores")
                nc.scalar.activation(scores, ps, AF.Sigmoid, scale=sv_ap)

                # scores_for_choice = scores + bias
                sb = wpool.tile([128, E], F32, tag="sb")
                nc.vector.tensor_add(sb, scores, bias_sb)
                sbg = sb.rearrange("p (g e) -> p g e", g=N_GROUP)

                # top-2 per group of 32 -> group scores
                g1 = wpool.tile([128, N_GROUP], F32, tag="g1")
                nc.vector.reduce_max(g1, sbg, axis=AX.X)
                kn = wpool.tile([128, E], F32, tag="kn")
                nc.vector.match_replace(out=kn, in_to_replace=g1, in_values=sb,
                                        imm_value=-1e30)
                g2 = wpool.tile([128, N_GROUP], F32, tag="g2")
                nc.vector.reduce_max(g2, kn.rearrange("p (g e) -> p g e", g=N_GROUP),
                                     axis=AX.X)
                gs = wpool.tile([128, N_GROUP], F32, tag="gs")
                nc.vector.tensor_add(gs, g1, g2)

                # top-4 groups: threshold at 4th largest of the 8 group scores
                g8 = wpool.tile([128, 8], F32, tag="g8")
                nc.vector.max(out=g8, in_=gs)
                gm = wpool.tile([128, N_GROUP], F32, tag="gm")
                nc.vector.tensor_scalar(gm, gs, g8[:, 3:4], None, op0=ALU.is_ge)

                # mask the biased scores and take top-8 (values + indices)
                tmp = wpool.tile([128, N_GROUP, EXP_PER_GROUP], F32, tag="tmp")
                nc.vector.tensor_mul(tmp, sbg,
                                     gm.unsqueeze(2).to_broadcast([128, N_GROUP, EXP_PER_GROUP]))
                tmpf = tmp.rearrange("p g e -> p (g e)")
                m8 = wpool.tile([128, 8], F32, tag="m8")
                i8 = wpool.tile([128, 8], U16, tag="i8")
                nc.vector.max_with_indices(m8, i8, tmpf)
                idx_t = wpool.tile([128, 8], U8, tag="idx")
                nc.vector.tensor_copy(idx_t, i8)

                # minimum boundary gap: min(top4-group cut, 8-vs-9 cut,
                # adjacent top-8 gaps) — the host repairs tokens where this
                # sits within the int16 quantization noise
                kn2 = wpool.tile([128, E], F32, tag="kn2")
                nc.vector.match_replace(out=kn2, in_to_replace=m8, in_values=tmpf,
                                        imm_value=-1e30)
                m9 = wpool.tile([128, 1], F32, tag="m9")
                nc.vector.reduce_max(m9, kn2.rearrange("p (a e) -> p a e", a=1),
                                     axis=AX.X)
                gapb = wpool.tile([128, 1], F32, tag="gapb")
                nc.vector.tensor_sub(gapb, m8[:, 7:8], m9)
                d7 = wpool.tile([128, 7], F32, tag="d7")
                nc.vector.tensor_sub(d7, m8[:, 0:7], m8[:, 1:8])
                gapa = wpool.tile([128, 1], F32, tag="gapa")
                nc.vector.tensor_reduce(gapa, d7.rearrange("p (a e) -> p a e", a=1),
                                        axis=AX.X, op=ALU.min)
                ggap = wpool.tile([128, 1], F32, tag="ggap")
                nc.vector.tensor_sub(ggap, g8[:, 3:4], g8[:, 4:5])
                gmin = wpool.tile([128, 1], F32, tag="gmin")
                nc.vector.tensor_tensor(gmin, gapb, gapa, op=ALU.min)
                gmin2 = wpool.tile([128, 1], F32, tag="gmin2")
                nc.vector.tensor_tensor(gmin2, gmin, ggap, op=ALU.min)

                m8h = wpool.tile([128, 8], F16, tag="m8h")
                nc.vector.tensor_copy(m8h, m8)
                nc.sync.dma_start(idx_out[i], idx_t)
                nc.sync.dma_start(m8_out[i], m8h)
                nc.sync.dma_start(gap_out[i], gmin2)

            # sweep 2 (chunks 8..15 + routing for the first HALF tiles),
            # interleaved with the remaining tiles' full runs so the PE
            # stays the pacing engine while routing drains behind it.
            for i in range(HALF):
                j = HALF + i
                if j + 2 < n_tiles:
                    splits[j + 2] = load_split(j + 2)
                xh_sb, xl_sb = splits.pop(i)
                mm_sweep(pss[i], xh_sb, xl_sb, H_CHUNKS // 2, H_CHUNKS,
                         False, True)
                finish_tile(i, pss.pop(i))
                if j < n_tiles:
                    psj = ppool.tile([128, E], F32, tag="ps")
                    xh2, xl2 = splits.pop(j)
                    mm_sweep(psj, xh2, xl2, 0, H_CHUNKS, True, True)
                    finish_tile(j, psj)
    return nc


def prep_shared(weight, bias_vec, s):
    ws = np.asarray(weight, np.float64) * 1024.0
    wh_ = ws.astype(np.float16)
    wl_ = (ws - wh_.astype(np.float64)).astype(np.float16)

    def tile_w(a):
        # [E, H] -> [H, E] -> [128p(h_inner), 16(h_outer), E]
        return np.ascontiguousarray(a.T.reshape(H_CHUNKS, 128, E).transpose(1, 0, 2))

    whl_ = np.concatenate([tile_w(wh_), tile_w(wl_)], axis=2)  # [128, 16, 512]
    bc = np.empty((1, E + 1), np.float32)
    bc[0, :E] = np.asarray(bias_vec, np.float32)
    bc[0, E] = (2.0 ** -10) / s
    return whl_, bc


_CACHED = {}


def _get_nc():
    if "nc" not in _CACHED:
        nc = bacc.Bacc("TRN2", num_devices=N_CORES)
        build_kernel(nc)
        nc.compile()
        _CACHED["nc"] = nc
    return _CACHED["nc"]


def make_in_maps(hidden_states, weight, e_score_correction_bias):
    x = np.asarray(hidden_states, np.float32).reshape(-1, H)
    s = np.float32(32700.0 / max(float(np.abs(x).max()), 1e-30))
    xi = np.rint(x * s).astype(np.int16)
    # natural [T, H] layout — no host transpose; the device DMA transposes
    xt = xi.reshape(N_CORES, N_TILES, 128, H)
    whl_, bc = prep_shared(np.asarray(weight, np.float32),
                           np.asarray(e_score_correction_bias, np.float32), s)
    return [{"xi": xt[c],
             "whl_part": np.ascontiguousarray(whl_[:, [c, 8 + c], :]),
             "bc": bc}
            for c in range(N_CORES)]


def _exact_routing(x_rows, weight, bias_vec):
    """Reference routing in numpy for a subset of tokens (f32, stable ties)."""
    logits = x_rows @ weight.T.astype(np.float32)
    scores = 1.0 / (1.0 + np.exp(-logits, dtype=np.float32))
    sc = scores + bias_vec[None, :]
    gs = np.sort(sc.reshape(-1, N_GROUP, EXP_PER_GROUP), axis=-1)[:, :, -2:].sum(-1)
    gsel = np.argsort(-gs, axis=-1, kind="stable")[:, :TOPK_GROUP_]
    gmask = np.zeros_like(gs)
    np.put_along_axis(gmask, gsel, 1.0, axis=-1)
    smask = np.repeat(gmask, EXP_PER_GROUP, axis=-1)
    tmp = np.where(smask > 0, sc, 0.0)
    tidx = np.argsort(-tmp, axis=-1, kind="stable")[:, :8].astype(np.int32)
    tw = np.take_along_axis(scores, tidx, axis=-1)
    return tidx, tw


TOPK_GROUP_ = 4
_SUS_THR = np.float32(3e-4)


def kernel(hidden_states, weight, e_score_correction_bias):
    in_maps = make_in_maps(hidden_states, weight, e_score_correction_bias)
    nc = _get_nc()
    res = bass_utils.run_bass_kernel_spmd(nc, in_maps, core_ids=list(range(N_CORES)))
    idx = np.concatenate([r["idx_out"].reshape(-1, 8) for r in res.results], axis=0)
    m8v = np.concatenate([r["m8_out"].reshape(-1, 8) for r in res.results], axis=0)
    gapv = np.concatenate([r["gap_out"].reshape(-1) for r in res.results], axis=0)
    idx = idx.astype(np.int32)
    # recover unbiased weights: wt_j = m8_j - bias[idx_j]
    bias_vec = np.asarray(e_score_correction_bias, np.float32)
    wt = m8v.astype(np.float32) - bias_vec[idx]

    # near-tie repair: tokens whose routing decisions sit within the int16
    # quantization noise of a boundary get recomputed exactly on host.
    sus = gapv < _SUS_THR
    if sus.any():
        x = np.asarray(hidden_states, np.float32).reshape(-1, H)
        w32 = np.asarray(weight, np.float32)
        tidx, tw = _exact_routing(x[sus], w32, bias_vec)
        idx[sus] = tidx
        wt[sus] = tw

    denom = wt.sum(axis=-1, keepdims=True) + np.float32(1e-20)
    wt = (wt / denom) * np.float32(2.5)
    return idx, wt.astype(np.float32)


# revision 44
# speedup vs baseline: 1.0169x; 1.0169x over previous
"""KimiMoEGate (sigmoid scoring, group-limited top-k) on 8 Trainium2 cores.

Strategy (hardcoded for hidden_states [4,4096,2048], weight [256,2048]):
  - Token-parallel: 16384 tokens sharded 2048/core across 8 cores.
  - x is uploaded as int16 fixed-point (2 B/elem — half of fp32): host sends
    xi = round(x * s) with s = 32700/max|x|. The device splits xi exactly into
    fp16 hi/lo (xh = f16(xi), xl = xi - xh; both exact since |xi| < 2^15 and
    the residual is a small integer), then computes router logits on PE as
    2 fp16 passes (xh*wh + xh*wl) plus one fp8 DoubleRow pass (xl*wh8 —
    xl's small integers are exact in fp8e4, and DoubleRow packs two
    h-chunks per matmul at 2 MACs/cell/cycle), accumulated in fp32 PSUM.
  - W (scaled by 1024, split into fp16 hi/lo on host) is NOT replicated over
    the wire: each core uploads 2 of the 16 h-chunks (262KB instead of 2.1MB)
    and the full slab is assembled on-device with a single fused AllGather
    into Shared HBM scratch.
  - bias and the runtime sigmoid scale 2^-10/s ride a single [1, 257] row,
    broadcast across partitions on-device with a K=1 matmul against ones.
  - ACT does the i16->f16 hi split + sigmoid; DVE does the lo split and the
    group-limited top-k (reduce_max / match_replace / max_with_indices).
  - x rides the wire in natural [T, H] layout; the XBAR DMA transposes each
    tile on the way into SBUF (h_inner on partitions), so the host does no
    transpose at all.
  - The device returns the top-8 indices (u8), the top-8 BIASED scores m8
    (f16) and one per-token f32 "minimum boundary gap" = min(top-4 group
    cut, 8-vs-9 cut, adjacent top-8 gaps). The host recovers weights as
    wt_j = m8_j - bias[idx_j] and normalizes (wt / (sum + 1e-20) * 2.5);
    tokens whose minimum gap sits within the int16 quantization noise
    (~2% of tokens) are recomputed exactly on the host, making the result
    match the reference almost exactly.
  - PE schedule: the first 8 tiles run chunks 0-7 as eight concurrent PSUM
    accumulation groups, then finish interleaved with the remaining tiles'
    full runs; splits are prefetched ahead and outputs DMA'd per tile, so
    only the last tile's short chain trails the PE.
"""

import numpy as np

from concourse import bacc, bass_utils
import concourse.mybir as mybir
from concourse.tile import TileContext

F16 = mybir.dt.float16
F32 = mybir.dt.float32
F8 = mybir.dt.float8e4
U16 = mybir.dt.uint16
U8 = mybir.dt.uint8
I16 = mybir.dt.int16
AF = mybir.ActivationFunctionType
ALU = mybir.AluOpType
AX = mybir.AxisListType

N_CORES = 8
N_GROUP = 8
EXP_PER_GROUP = 32
E = 256
H = 2048
H_CHUNKS = 16  # 2048 / 128
T_TOTAL = 16384
T_CORE = T_TOTAL // N_CORES
N_TILES = T_CORE // 128  # 16


def build_kernel(nc, n_tiles=N_TILES):
    # natural token-major layout; the XBAR DMA transposes tiles on the way in
    xi = nc.dram_tensor("xi", [n_tiles, 128, H], I16, kind="ExternalInput").ap()
    # per-core W slice: [:, 0, :] = h-chunk c, [:, 1, :] = h-chunk 8+c
    # (chunk layout [128p(h_inner), 2E] with cols 0:256 = W_hi, 256:512 = W_lo)
    whl_part = nc.dram_tensor("whl_part", [128, 2, 2 * E], F16, kind="ExternalInput").ap()
    # bias row + sigmoid scale: bc[0, :256] = bias, bc[0, 256] = 2^-10/s
    bc = nc.dram_tensor("bc", [1, E + 1], F32, kind="ExternalInput").ap()
    idx_out = nc.dram_tensor("idx_out", [n_tiles, 128, 8], U8, kind="ExternalOutput").ap()
    m8_out = nc.dram_tensor("m8_out", [n_tiles, 128, 8], F16, kind="ExternalOutput").ap()
    # minimum routing-boundary gap per token, for host-side near-tie repair
    gap_out = nc.dram_tensor("gap_out", [n_tiles, 128, 1], F32, kind="ExternalOutput").ap()

    with TileContext(nc) as tc:
        with (
            tc.tile_pool(name="const", bufs=1) as cpool,
            tc.tile_pool(name="xin", bufs=4) as xipool,
            tc.tile_pool(name="xsplit", bufs=12) as xpool,
            tc.tile_pool(name="work", bufs=3) as wpool,
            tc.tile_pool(name="psum", bufs=8, space="PSUM") as ppool,
            tc.tile_pool(name="dram", bufs=1, space="DRAM") as dpool,
        ):
            # ---- W assembly: two pipelined AllGathers (chunks 0-7, 8-15) ----
            whl_sb = cpool.tile([128, H_CHUNKS, 2 * E], F16)
            # single fused gather: one collective launch for the whole 2.1MB
            # (two serialized collectives pay the ~55us launch+latency twice)
            gall = dpool.tile([128, 2, 2 * E], F16)
            goutall = nc.dram_tensor("goutall", [N_CORES, 128, 2, 2 * E], F16,
                                     kind="Internal", addr_space="Shared").ap()
            nc.gpsimd.dma_start(gall[:], whl_part[:])
            nc.gpsimd.collective_compute(
                "AllGather", ALU.bypass,
                replica_groups=[list(range(N_CORES))],
                ins=[gall.opt()], outs=[goutall.opt()])

            # ---- bias/scale broadcast across partitions via K=1 matmul ----
            bc_sb = cpool.tile([1, E + 1], F32)
            nc.sync.dma_start(bc_sb, bc)
            ones = cpool.tile([1, 128], F32)
            nc.gpsimd.memset(ones, 1.0)
            bias_full = cpool.tile([128, E], F32)
            ps_b = ppool.tile([128, E], F32, tag="ps")
            nc.tensor.matmul(ps_b, ones, bc_sb[:, :E], start=True, stop=True)
            nc.vector.tensor_copy(bias_full, ps_b)
            bias_sb = bias_full[:, :E]
            sv_sb = cpool.tile([128, 1], F32)
            ps_v = ppool.tile([128, 1], F32, tag="ps")
            nc.tensor.matmul(ps_v, ones, bc_sb[:, E:E + 1], start=True, stop=True)
            nc.vector.tensor_copy(sv_sb, ps_v)
            sv_ap = sv_sb[:, 0:1]

            # Preload the sigmoid activation table during the initial DMA
            # dead time so it isn't loaded mid-pipeline.
            dummy = cpool.tile([128, 1], F32)
            dummy2 = cpool.tile([128, 1], F32)
            nc.gpsimd.memset(dummy, 0.0)
            nc.scalar.activation(dummy2, dummy, AF.Sigmoid)

            def load_split(i):
                # exact split: xh = f16(xi) (RNE), xl = xi - xh (small int,
                # exact in f16)
                xi_sb = xipool.tile([128, H_CHUNKS, 128], I16, tag="xi")
                nc.sync.dma_start_transpose(xi_sb, xi[i])
                xh_sb = xpool.tile([128, H_CHUNKS, 128], F16, tag="xh")
                nc.scalar.activation(xh_sb, xi_sb, AF.Copy)
                # xl is a small integer (|xl| <= 8) — exact in fp8e4, which
                # lets the xl pass run as DoubleRow fp8 matmuls
                xl_sb = xpool.tile([128, H_CHUNKS, 128], F8, tag="xl")
                nc.gpsimd.tensor_sub(xl_sb, xi_sb, xh_sb)
                return xh_sb, xl_sb

            # splits for the first HALF tiles are issued BEFORE the whl SBUF
            # loads: their xi DMAs are ready immediately and run during the
            # gathers instead of queueing behind DMA ops that wait on them.
            HALF = min(8, n_tiles)
            splits = {i: load_split(i) for i in range(HALF)}
            # fp8 copy of W_hi for the DoubleRow xl pass (one-time, post-gather)
            wh8_sb = cpool.tile([128, H_CHUNKS, E], F8)
            for ho in range(H_CHUNKS):
                nc.sync.dma_start(whl_sb[:, ho], goutall[ho % 8][:, ho // 8, :])
                nc.vector.tensor_copy(wh8_sb[:, ho], whl_sb[:, ho, :E])

            def mm_sweep(ps, xh_sb, xl_sb, ho_lo, ho_hi, first, last):
                for ho in range(ho_lo, ho_hi):
                    nc.tensor.matmul(ps, xh_sb[:, ho, :], whl_sb[:, ho, :E],
                                     start=(first and ho == ho_lo), stop=False)
                    nc.tensor.matmul(ps, xh_sb[:, ho, :], whl_sb[:, ho, E:],
                                     start=False, stop=False)
                # xl pass: fp8 DoubleRow, two h-chunks per matmul
                for ho in range(ho_lo, ho_hi, 2):
                    nc.tensor.matmul(ps, xl_sb[:, ho:ho + 2, :],
                                     wh8_sb[:, ho:ho + 2, :],
                                     perf_mode=mybir.MatmulPerfMode.DoubleRow,
                                     start=False,
                                     stop=(last and ho == ho_hi - 2))

            # sweep 1: first HALF tiles x chunks 0..7 — PSUM holds 8 open
            # accumulation groups, so these run as soon as gather A lands,
            # entirely under gather B.
            pss = {}
            for i in range(HALF):
                ps_i = ppool.tile([128, E], F32, tag="ps", name=f"ps{i}")
                pss[i] = ps_i
                mm_sweep(ps_i, *splits[i], 0, H_CHUNKS // 2, True, False)

            for j in range(HALF, min(HALF + 2, n_tiles)):
                splits[j] = load_split(j)

            def finish_tile(i, ps):
                # scores = sigmoid(logits); psum holds 1024*s*logits
                scores = wpool.tile([128, E], F32, tag="scores")
                nc.scalar.activation(scores, ps, AF.Sigmoid, scale=sv_ap)

                # scores_for_choice = scores + bias
                sb = wpool.tile([128, E], F32, tag="sb")
                nc.vector.tensor_add(sb, scores, bias_sb)
                sbg = sb.rearrange("p (g e) -> p g e", g=N_GROUP)

                # top-2 per group of 32 -> group scores
                g1 = wpool.tile([128, N_GROUP], F32, tag="g1")
                nc.vector.reduce_max(g1, sbg, axis=AX.X)
                kn = wpool.tile([128, E], F32, tag="kn")
                nc.vector.match_replace(out=kn, in_to_replace=g1, in_values=sb,
                                        imm_value=-1e30)
                g2 = wpool.tile([128, N_GROUP], F32, tag="g2")
                nc.vector.reduce_max(g2, kn.rearrange("p (g e) -> p g e", g=N_GROUP),
                                     axis=AX.X)
                gs = wpool.tile([128, N_GROUP], F32, tag="gs")
                nc.vector.tensor_add(gs, g1, g2)

                # top-4 groups: threshold at 4th largest of the 8 group scores
                g8 = wpool.tile([128, 8], F32, tag="g8")
                nc.vector.max(out=g8, in_=gs)
                gm = wpool.tile([128, N_GROUP], F32, tag="gm")
                nc.vector.tensor_scalar(gm, gs, g8[:, 3:4], None, op0=ALU.is_ge)

                # mask the biased scores and take top-8 (values + indices)
                tmp = wpool.tile([128, N_GROUP, EXP_PER_GROUP], F32, tag="tmp")
                nc.vector.tensor_mul(tmp, sbg,
                                     gm.unsqueeze(2).to_broadcast([128, N_GROUP, EXP_PER_GROUP]))
                tmpf = tmp.rearrange("p g e -> p (g e)")
                m8 = wpool.tile([128, 8], F32, tag="m8")
                i8 = wpool.tile([128, 8], U16, tag="i8")
                nc.vector.max_with_indices(m8, i8, tmpf)
                idx_t = wpool.tile([128, 8], U8, tag="idx")
                nc.scalar.copy(idx_t, i8)

                # minimum boundary gap: min(top4-group cut, 8-vs-9 cut,
                # adjacent top-8 gaps) — the host repairs tokens where this
                # sits within the int16 quantization noise
                kn2 = wpool.tile([128, E], F32, tag="kn2")
                nc.vector.match_replace(out=kn2, in_to_replace=m8, in_values=tmpf,
                                        imm_value=-1e30)
                m9 = wpool.tile([128, 1], F32, tag="m9")
                nc.vector.reduce_max(m9, kn2.rearrange("p (a e) -> p a e", a=1),
                                     axis=AX.X)
                gapb = wpool.tile([128, 1], F32, tag="gapb")
                nc.vector.tensor_sub(gapb, m8[:, 7:8], m9)
                d7 = wpool.tile([128, 7], F32, tag="d7")
                nc.vector.tensor_sub(d7, m8[:, 0:7], m8[:, 1:8])
                gapa = wpool.tile([128, 1], F32, tag="gapa")
                nc.vector.tensor_reduce(gapa, d7.rearrange("p (a e) -> p a e", a=1),
                                        axis=AX.X, op=ALU.min)
                ggap = wpool.tile([128, 1], F32, tag="ggap")
                nc.vector.tensor_sub(ggap, g8[:, 3:4], g8[:, 4:5])
                gmin = wpool.tile([128, 1], F32, tag="gmin")
                nc.vector.tensor_tensor(gmin, gapb, gapa, op=ALU.min)
                gmin2 = wpool.tile([128, 1], F32, tag="gmin2")
                nc.vector.tensor_tensor(gmin2, gmin, ggap, op=ALU.min)

                m8h = wpool.tile([128, 8], F16, tag="m8h")
                nc.scalar.copy(m8h, m8)
                nc.sync.dma_start(idx_out[i], idx_t)
                nc.sync.dma_start(m8_out[i], m8h)
                nc.sync.dma_start(gap_out[i], gmin2)

            # sweep 2 (chunks 8..15 + routing for the first HALF tiles),
            # interleaved with the remaining tiles' full runs so the PE
            # stays the pacing engine while routing drains behind it.
            for i in range(HALF):
                j = HALF + i
                if j + 2 < n_tiles:
                    splits[j + 2] = load_split(j + 2)
                xh_sb, xl_sb = splits.pop(i)
                mm_sweep(pss[i], xh_sb, xl_sb, H_CHUNKS // 2, H_CHUNKS,
                         False, True)
                finish_tile(i, pss.pop(i))
                if j < n_tiles:
                    psj = ppool.tile([128, E], F32, tag="ps")
                    xh2, xl2 = splits.pop(j)
                    mm_sweep(psj, xh2, xl2, 0, H_CHUNKS, True, True)
                    finish_tile(j, psj)
    return nc


def prep_shared(weight, bias_vec, s):
    ws = np.asarray(weight, np.float64) * 1024.0
    wh_ = ws.astype(np.float16)
    wl_ = (ws - wh_.astype(np.float64)).astype(np.float16)

    def tile_w(a):
        # [E, H] -> [H, E] -> [128p(h_inner), 16(h_outer), E]
        return np.ascontiguousarray(a.T.reshape(H_CHUNKS, 128, E).transpose(1, 0, 2))

    whl_ = np.concatenate([tile_w(wh_), tile_w(wl_)], axis=2)  # [128, 16, 512]
    bc = np.empty((1, E + 1), np.float32)
    bc[0, :E] = np.asarray(bias_vec, np.float32)
    bc[0, E] = (2.0 ** -10) / s
    return whl_, bc


_CACHED = {}


def _get_nc():
    if "nc" not in _CACHED:
        nc = bacc.Bacc("TRN2", num_devices=N_CORES)
        build_kernel(nc)
        nc.compile()
        _CACHED["nc"] = nc
    return _CACHED["nc"]


def make_in_maps(hidden_states, weight, e_score_correction_bias):
    x = np.asarray(hidden_states, np.float32).reshape(-1, H)
    s = np.float32(32700.0 / max(float(np.abs(x).max()), 1e-30))
    xi = np.rint(x * s).astype(np.int16)
    # natural [T, H] layout — no host transpose; the device DMA transposes
    xt = xi.reshape(N_CORES, N_TILES, 128, H)
    whl_, bc = prep_shared(np.asarray(weight, np.float32),
                           np.asarray(e_score_correction_bias, np.float32), s)
    return [{"xi": xt[c],
             "whl_part": np.ascontiguousarray(whl_[:, [c, 8 + c], :]),
             "bc": bc}
            for c in range(N_CORES)]


def _exact_routing(x_rows, weight, bias_vec):
    """Reference routing in numpy for a subset of tokens (f32, stable ties)."""
    logits = x_rows @ weight.T.astype(np.float32)
    scores = 1.0 / (1.0 + np.exp(-logits, dtype=np.float32))
    sc = scores + bias_vec[None, :]
    gs = np.sort(sc.reshape(-1, N_GROUP, EXP_PER_GROUP), axis=-1)[:, :, -2:].sum(-1)
    gsel = np.argsort(-gs, axis=-1, kind="stable")[:, :TOPK_GROUP_]
    gmask = np.zeros_like(gs)
    np.put_along_axis(gmask, gsel, 1.0, axis=-1)
    smask = np.repeat(gmask, EXP_PER_GROUP, axis=-1)
    tmp = np.where(smask > 0, sc, 0.0)
    tidx = np.argsort(-tmp, axis=-1, kind="stable")[:, :8].astype(np.int32)
    tw = np.take_along_axis(scores, tidx, axis=-1)
    return tidx, tw


TOPK_GROUP_ = 4
_SUS_THR = np.float32(3e-4)


def kernel(hidden_states, weight, e_score_correction_bias):
    in_maps = make_in_maps(hidden_states, weight, e_score_correction_bias)
    nc = _get_nc()
    res = bass_utils.run_bass_kernel_spmd(nc, in_maps, core_ids=list(range(N_CORES)))
    idx = np.concatenate([r["idx_out"].reshape(-1, 8) for r in res.results], axis=0)
    m8v = np.concatenate([r["m8_out"].reshape(-1, 8) for r in res.results], axis=0)
    gapv = np.concatenate([r["gap_out"].reshape(-1) for r in res.results], axis=0)
    idx = idx.astype(np.int32)
    # recover unbiased weights: wt_j = m8_j - bias[idx_j]
    bias_vec = np.asarray(e_score_correction_bias, np.float32)
    wt = m8v.astype(np.float32) - bias_vec[idx]

    # near-tie repair: tokens whose routing decisions sit within the int16
    # quantization noise of a boundary get recomputed exactly on host.
    sus = gapv < _SUS_THR
    if sus.any():
        x = np.asarray(hidden_states, np.float32).reshape(-1, H)
        w32 = np.asarray(weight, np.float32)
        tidx, tw = _exact_routing(x[sus], w32, bias_vec)
        idx[sus] = tidx
        wt[sus] = tw

    denom = wt.sum(axis=-1, keepdims=True) + np.float32(1e-20)
    wt = (wt / denom) * np.float32(2.5)
    return idx, wt.astype(np.float32)
